# revision 1
# baseline (speedup 1.0000x reference)
"""PointGNN Trainium2 kernel (nn_PointGNN_11931419149118).

Algebraic collapse of the reference: the edge-MLP input is
concat(zeros(3), adj ? state[j] : 0), so for adjacent (i,j) the edge
feature E[j] = MLP_f([0, state[j]]) depends only on j. Since MLP_f ends
in a ReLU and e is re-masked by adj before the max over j,
    agg[i, c] = max_j adj[i, j] * E'[j, c]        (E' = pre-relu edge MLP)
where the zeros contributed by non-neighbors supply the final ReLU for
free (max(0, .) == relu, and every point has non-neighbors). This
avoids materializing the reference's (N, M, M, 128) tensors entirely.

Mapping: the masked max runs on the vector engine in fp16 as
group-batched mult ops (2 elem/cyc) + a pairwise-max tree + small tail
reduce; E' rows are broadcast across partitions by tensor-engine
"selector" matmuls (lhsT = e_c x ones, a zero-stride AP view of an
identity tile); the scalar engine converts PSUM results to fp16 SBUF.
MLPs run fp32 in transposed layout (channels on partitions) so weights
load as natural (K, N) lhsT tiles and biases are per-partition columns.

Sharding (8 cores): cores [4q, 4q+4) own frames {2q, 2q+1}, each core
taking a 32-channel slice of the 128 edge channels for BOTH frames.
The two frames interleave: while one frame's agg AllGather (4-wide) and
state update run, the other frame's masked max keeps the vector engine
busy, hiding the per-timestep communication latency.
"""

import sys
import types

sys.path.insert(0, "/opt/trn_rl_repo")

import numpy as np
from contextlib import ExitStack

import concourse.bass as bass
import concourse.mybir as mybir
import concourse.tile as tile
from concourse import bacc
from concourse.bass_utils import run_bass_kernel_spmd
from concourse.masks import make_identity

F32 = mybir.dt.float32
F16 = mybir.dt.float16
AF = mybir.ActivationFunctionType
ALU = mybir.AluOpType
AX = mybir.AxisListType

N_FRAMES = 4
M = 384          # points per frame
P = 128          # partitions
NB = M // P      # 3 destination blocks
T = 3            # timesteps
C = 128          # edge channels
NS = 2           # frame streams per core
CH = C // 4      # channels per core (quarter)
G = 8            # channel group size for batched DVE ops
NG = CH // G     # groups per core per stream
R = 0.05         # squared-distance threshold
N_CORES = 8
REPLICA_GROUPS = [[0, 1, 2, 3], [4, 5, 6, 7]]


def _register_ntff_hook():
    """Register the axon NTFF profile hook the image's antenv lacks."""
    try:
        import antenv
        if "antenv.axon_hooks" in sys.modules:
            return
        mod = types.ModuleType("antenv.axon_hooks")
        _hook = [None]
        mod.set_axon_ntff_profile_hook = lambda h: _hook.__setitem__(0, h)
        mod.get_axon_ntff_profile_hook = lambda: _hook[0]
        sys.modules["antenv.axon_hooks"] = mod
        antenv.axon_hooks = mod
        from trn_agent_boot.trn_boot import _ntff_profile_via_ctypes
        mod.set_axon_ntff_profile_hook(
            _ntff_profile_via_ctypes("/opt/axon/libaxon_pjrt.so")
        )
    except Exception:
        pass


def _load_col(nc, pool, dram_ap, p, tag):
    """Load a length-p 1D DRAM vector as a (p, 1) SBUF column."""
    col = pool.tile([p, 1], F32, tag=tag, name=tag)
    nc.sync.dma_start(out=col, in_=dram_ap.rearrange("(n one) -> n one", one=1))
    return col


def build(ctx, tc):
    nc = tc.nc

    x_in = nc.declare_dram_parameter("x", [NS, M, 3], F32, isOutput=False)
    wspec = {
        "fW1s": (T, 3, 64), "fb1": (T, 64),
        "fW2": (T, 64, C), "fb2": (T, C),
        "fW3c": (T, C, CH), "fb3c": (T, CH),
        "gb1": (T, 64),
        "gW2": (T, 64, 32), "gb2": (T, 32),
        "gW3": (T, 32, 3), "gb3": (T, 3),
    }
    w = {
        name: nc.declare_dram_parameter(name, list(shp), F32, isOutput=False)
        for name, shp in wspec.items()
    }
    w["gW1"] = nc.declare_dram_parameter("gW1", [T, C, 64], F16, isOutput=False)
    out_ext = nc.declare_dram_parameter("out", [NS, M, 3], F32, isOutput=True)

    agg_out = [[nc.dram_tensor(f"agg_out_t{t}s{s}", [CH, M], F16)
                for s in range(NS)] for t in range(T)]
    agg_full = [[nc.dram_tensor(f"agg_full_t{t}s{s}", [4, CH, M], F16)
                 for s in range(NS)] for t in range(T)]

    consts = ctx.enter_context(tc.tile_pool(name="consts", bufs=1))
    scratch_pool = ctx.enter_context(tc.tile_pool(name="scratch", bufs=3))
    work = ctx.enter_context(tc.tile_pool(name="work", bufs=2))
    ebc_pool = ctx.enter_context(tc.tile_pool(name="ebc", bufs=3))
    mg_pool = ctx.enter_context(tc.tile_pool(name="mg", bufs=3))
    psum = ctx.enter_context(
        tc.tile_pool(name="psum", bufs=2, space=bass.MemorySpace.PSUM)
    )
    psum_bc = ctx.enter_context(
        tc.tile_pool(name="psum_bc", bufs=2, space=bass.MemorySpace.PSUM)
    )
    psum_g = ctx.enter_context(
        tc.tile_pool(name="psum_g", bufs=1, space=bass.MemorySpace.PSUM)
    )

    identity = consts.tile([P, P], F32, tag="identity")
    make_identity(nc, identity)
    identity16 = consts.tile([P, P], F16, tag="identity16")
    make_identity(nc, identity16)

    def sel16(c, k):
        col = identity16[:k, c:c + 1]
        return bass.AP(col.tensor, col.offset, [list(col.ap[0]), [0, P]])

    # ---- per-stream: load x, adjacency ----
    xTs, adjreps = [], []
    for s in range(NS):
        xn = []
        xT = consts.tile([3, M], F32, tag=f"xT{s}", name=f"xT{s}")
        for ib in range(NB):
            xn_ib = consts.tile([P, 3], F32, tag=f"xn{s}_{ib}",
                                name=f"xn{s}_{ib}")
            nc.sync.dma_start(out=xn_ib, in_=x_in[s, ib * P:(ib + 1) * P, :])
            xn.append(xn_ib)
            ps = psum.tile([3, P], F32, tag="aux", name=f"xt_ps{s}_{ib}")
            nc.tensor.transpose(ps, xn_ib, identity)
            nc.scalar.copy(out=xT[:, ib * P:(ib + 1) * P], in_=ps)
        xTs.append(xT)

        # adjacency, diff-based (not Gram) to dodge cancellation near R
        bcx = []
        for d in range(3):
            ps = psum.tile([P, M], F32, tag="aux", name=f"bcx_ps{s}_{d}")
            col = identity[:3, d:d + 1]
            sel3 = bass.AP(col.tensor, col.offset,
                           [list(col.ap[0]), [0, P]])
            nc.tensor.matmul(ps, sel3, xT, start=True, stop=True)
            b = scratch_pool.tile([P, M], F32, tag="bcx", name=f"bcx{s}_{d}")
            nc.scalar.copy(out=b, in_=ps)
            bcx.append(b)
        adjrep = []
        for ib in range(NB):
            acc = scratch_pool.tile([P, M], F32, tag="adj_acc")
            for d in range(3):
                dif = scratch_pool.tile([P, M], F32, tag="adj_dif")
                nc.vector.tensor_scalar(
                    out=dif, in0=bcx[d], scalar1=xn[ib][:, d:d + 1],
                    scalar2=None, op0=ALU.subtract,
                )
                if d == 0:
                    nc.vector.tensor_mul(acc, dif, dif)
                else:
                    sq = scratch_pool.tile([P, M], F32, tag="adj_sq")
                    nc.vector.tensor_mul(sq, dif, dif)
                    nc.vector.tensor_add(acc, acc, sq)
            a16 = scratch_pool.tile([P, M], F16, tag="adj16",
                                    name=f"adj{s}_{ib}")
            nc.vector.tensor_scalar(
                out=a16, in0=acc, scalar1=R, scalar2=None, op0=ALU.is_lt,
            )
            rep = consts.tile([P, G, M], F16, tag=f"adjrep{s}_{ib}",
                              name=f"adjrep{s}_{ib}")
            a_b = bass.AP(a16.tensor, a16.offset,
                          [list(a16.ap[0]), [0, G], list(a16.ap[1])])
            nc.vector.tensor_copy(rep, a_b)
            adjrep.append(rep)
        adjreps.append(adjrep)

    # ---- weight/bias tiles ----
    wt = {}
    for t in range(T):
        for ck in range(4):
            tl = consts.tile([C // 4, 64], F16, tag=f"gW1c{ck}_{t}",
                             name=f"gW1c{ck}_{t}")
            nc.sync.dma_start(
                out=tl, in_=w["gW1"][t, ck * (C // 4):(ck + 1) * (C // 4), :])
            wt[("gW1c", t, ck)] = tl
        for name, shp in wspec.items():
            if len(shp) == 3:
                tl = consts.tile([shp[1], shp[2]], F32, tag=f"{name}{t}",
                                 name=f"{name}{t}")
                nc.sync.dma_start(out=tl, in_=w[name][t])
            else:
                tl = _load_col(nc, consts, w[name][t], shp[1], f"{name}{t}")
            wt[(name, t)] = tl

    def mlp_layer(rhs, wname, bname, t, ndim, relu=True, out_dtype=F32,
                  nm=""):
        ps = psum.tile([ndim, M], F32, tag="mlp", name=f"ps_{nm}")
        nc.tensor.matmul(ps, wt[(wname, t)], rhs, start=True, stop=True)
        o = work.tile([ndim, M], out_dtype, tag=f"act_{wname}", name=nm)
        nc.scalar.activation(
            out=o, in_=ps, func=AF.Relu if relu else AF.Identity,
            bias=wt[(bname, t)], scale=1.0,
        )
        return o

    states = list(xTs)

    def compute_phase(s, t):
        """edge MLP + masked max + transpose + AllGather launch."""
        stateT = states[s]
        h1T = mlp_layer(stateT, "fW1s", "fb1", t, 64, nm=f"h1_{s}_{t}")
        h2T = mlp_layer(h1T, "fW2", "fb2", t, C, nm=f"h2_{s}_{t}")
        ET = mlp_layer(h2T, "fW3c", "fb3c", t, CH, relu=False,
                       out_dtype=F16, nm=f"ET_{s}_{t}")

        aggblk = work.tile([P, NB, CH], F16, tag="aggblk",
                           name=f"aggblk{s}_{t}")
        for cg in range(NG):
            ebcg = ebc_pool.tile([P, G, M], F16, tag="ebcg")
            for cc in range(G):
                ps = psum_bc.tile([P, M], F32, tag="ebc",
                                  name=f"ebc{t}_{s}_{cg}_{cc}")
                nc.tensor.matmul(ps, sel16(cg * G + cc, CH), ET,
                                 start=True, stop=True)
                nc.scalar.copy(out=ebcg[:, cc, :], in_=ps)
            mg = mg_pool.tile([P, NB, G, M], F16, tag="mgrp")
            mg2 = mg_pool.tile([P, NB, G, M // 2], F16, tag="mgrp2")
            for ib in range(NB):
                nc.vector.tensor_mul(mg[:, ib], adjreps[s][ib], ebcg)
            nc.vector.tensor_tensor(
                out=mg2, in0=mg[:, :, :, :192], in1=mg[:, :, :, 192:],
                op=ALU.max)
            nc.vector.tensor_tensor(
                out=mg[:, :, :, :96], in0=mg2[:, :, :, :96],
                in1=mg2[:, :, :, 96:], op=ALU.max)
            nc.vector.tensor_tensor(
                out=mg2[:, :, :, :48], in0=mg[:, :, :, :48],
                in1=mg[:, :, :, 48:96], op=ALU.max)
            nc.vector.tensor_reduce(
                out=aggblk[:, :, cg * G:(cg + 1) * G],
                in_=mg2[:, :, :, :48], axis=AX.X, op=ALU.max,
            )

        aggT = work.tile([CH, M], F16, tag="aggT", name=f"aggT{s}_{t}")
        for ib in range(NB):
            ps = psum.tile([CH, P], F16, tag="aux", name=f"tr_agg{t}_{s}_{ib}")
            nc.tensor.transpose(ps, aggblk[:, ib, :], identity16)
            nc.scalar.copy(out=aggT[:, ib * P:(ib + 1) * P], in_=ps)
        nc.sync.dma_start(out=agg_out[t][s][:], in_=aggT)
        nc.gpsimd.collective_compute(
            "AllGather", ALU.bypass, replica_groups=REPLICA_GROUPS,
            ins=[agg_out[t][s][:]], outs=[agg_full[t][s][:]],
        )

    def finish_phase(s, t):
        """gather in, MLP_g (layer 1 as per-chunk K accumulation), state."""
        ps_g1 = psum_g.tile([64, M], F32, tag=f"psg1_{s}", name=f"psg1_{s}{t}")
        for r in range(4):
            part = work.tile([CH, M], F16, tag=f"aggpart{s}_{r}",
                             name=f"aggpart{t}_{s}_{r}")
            nc.sync.dma_start(out=part, in_=agg_full[t][s][r])
            nc.tensor.matmul(ps_g1, wt[("gW1c", t, r)], part,
                             start=(r == 0), stop=(r == 3))
        g1T = work.tile([64, M], F32, tag="g1T", name=f"g1T_{s}_{t}")
        nc.scalar.activation(out=g1T, in_=ps_g1, func=AF.Relu,
                             bias=wt[("gb1", t)], scale=1.0)
        g2T = mlp_layer(g1T, "gW2", "gb2", t, 32, nm=f"g2_{s}_{t}")
        gdT = mlp_layer(g2T, "gW3", "gb3", t, 3, nm=f"g3_{s}_{t}")
        newT = work.tile([3, M], F32, tag=f"stateT{s}", name=f"stateT{s}_{t}")
        nc.vector.tensor_add(newT, gdT, states[s])
        states[s] = newT

    for t in range(T):
        for s in range(NS):
            compute_phase(s, t)
        for s in range(NS):
            finish_phase(s, t)

    # ---- write out: transpose states back to (M, 3) ----
    for s in range(NS):
        for ib in range(NB):
            ps = psum.tile([P, 3], F32, tag="aux", name=f"tr_out{s}_{ib}")
            nc.tensor.transpose(ps, states[s][:, ib * P:(ib + 1) * P],
                                identity[:3, :3])
            o = work.tile([P, 3], F32, tag="out_sb", name=f"out_sb{s}_{ib}")
            nc.scalar.copy(out=o, in_=ps)
            nc.sync.dma_start(out=out_ext[s, ib * P:(ib + 1) * P, :], in_=o)


_NC_CACHE = None


def _build_nc():
    global _NC_CACHE
    if _NC_CACHE is None:
        nc = bacc.Bacc(
            "TRN2", target_bir_lowering=False, debug=False,
            num_devices=N_CORES,
        )
        with ExitStack() as ctx:
            tc = ctx.enter_context(tile.TileContext(nc))
            build(ctx, tc)
        nc.compile()
        _NC_CACHE = nc
    return _NC_CACHE


def _in_maps(inputs):
    maps = []
    fW1s = np.ascontiguousarray(inputs["fW1"][:, 3:6, :])
    for k in range(N_CORES):
        q, r = k // 4, k % 4
        sl = slice(CH * r, CH * r + CH)
        maps.append({
            "x": np.ascontiguousarray(inputs["x"][2 * q:2 * q + 2]),
            "fW1s": fW1s,
            "fb1": inputs["fb1"],
            "fW2": inputs["fW2"],
            "fb2": inputs["fb2"],
            "fW3c": np.ascontiguousarray(inputs["fW3"][:, :, sl]),
            "fb3c": np.ascontiguousarray(inputs["fb3"][:, sl]),
            "gW1": inputs["gW1"].astype(np.float16),
            "gb1": inputs["gb1"],
            "gW2": inputs["gW2"],
            "gb2": inputs["gb2"],
            "gW3": inputs["gW3"],
            "gb3": inputs["gb3"],
        })
    return maps


def kernel(trace=False, **inputs):
    _register_ntff_hook()
    nc = _build_nc()
    inputs = {k: np.asarray(v, np.float32) for k, v in inputs.items()}
    res = run_bass_kernel_spmd(
        nc, _in_maps(inputs), list(range(N_CORES)), trace=trace,
    )
    out = np.stack([res.results[4 * (f // 2)]["out"][f % 2]
                    for f in range(N_FRAMES)])
    if trace:
        kernel.last_results = res
    return out.astype(np.float32)



# revision 5
# speedup vs baseline: 1.1217x; 1.1217x over previous
"""PointGNN Trainium2 kernel (nn_PointGNN_11931419149118).

Algebraic collapse of the reference: the edge-MLP input is
concat(zeros(3), adj ? state[j] : 0), so for adjacent (i,j) the edge
feature E[j] = MLP_f([0, state[j]]) depends only on j. Since MLP_f ends
in a ReLU and e is re-masked by adj before the max over j,
    agg[i, c] = max_j adj[i, j] * E'[j, c]        (E' = pre-relu edge MLP)
where the zeros contributed by non-neighbors supply the final ReLU for
free (max(0, .) == relu, and every point has non-neighbors). This
avoids materializing the reference's (N, M, M, 128) tensors entirely.

Mapping: the masked max runs on the vector engine in fp16 as one
group-batched mult (adjacency broadcast across channel groups via
0-stride APs) + a pairwise-max tree ending in a narrow reduce; E' rows
are broadcast across partitions by tensor-engine "selector" matmuls
(lhsT = e_c x ones, a zero-stride AP view of an identity tile); the
scalar engine converts PSUM results to fp16 SBUF.

Sharding (8 cores): cores [4q, 4q+4) own frames {2q, 2q+1}, each core
taking a 32-channel slice of the 128 edge channels for BOTH frames.
The two frames are software-pipelined: stream s's agg AllGather, MLP_g
(whose state update is folded into the g3 PSUM via an identity-matmul
accumulate, keeping it off the vector engine), and the next timestep's
edge MLP + broadcasts all execute under the other stream's ~40us of
masked-max vector work, so the vector engine never idles between
timesteps.
"""

import sys
import types

sys.path.insert(0, "/opt/trn_rl_repo")

import numpy as np
from contextlib import ExitStack

import concourse.bass as bass
import concourse.mybir as mybir
import concourse.tile as tile
from concourse import bacc
from concourse.bass_utils import run_bass_kernel_spmd
from concourse.masks import make_identity

F32 = mybir.dt.float32
F16 = mybir.dt.float16
AF = mybir.ActivationFunctionType
ALU = mybir.AluOpType
AX = mybir.AxisListType

N_FRAMES = 4
M = 384          # points per frame
P = 128          # partitions
NB = M // P      # 3 destination blocks
T = 3            # timesteps
C = 128          # edge channels
NS = 2           # frame streams per core
CH = C // 4      # channels per core (quarter)
G = 8            # channel group size for batched DVE ops
NG = CH // G     # groups per core per stream
R = 0.05         # squared-distance threshold
N_CORES = 8
REPLICA_GROUPS = [[0, 1, 2, 3], [4, 5, 6, 7]]


def _register_ntff_hook():
    """Register the axon NTFF profile hook the image's antenv lacks."""
    try:
        import antenv
        if "antenv.axon_hooks" in sys.modules:
            return
        mod = types.ModuleType("antenv.axon_hooks")
        _hook = [None]
        mod.set_axon_ntff_profile_hook = lambda h: _hook.__setitem__(0, h)
        mod.get_axon_ntff_profile_hook = lambda: _hook[0]
        sys.modules["antenv.axon_hooks"] = mod
        antenv.axon_hooks = mod
        from trn_agent_boot.trn_boot import _ntff_profile_via_ctypes
        mod.set_axon_ntff_profile_hook(
            _ntff_profile_via_ctypes("/opt/axon/libaxon_pjrt.so")
        )
    except Exception:
        pass


def _load_col(nc, pool, dram_ap, p, tag):
    """Load a length-p 1D DRAM vector as a (p, 1) SBUF column."""
    col = pool.tile([p, 1], F32, tag=tag, name=tag)
    nc.sync.dma_start(out=col, in_=dram_ap.rearrange("(n one) -> n one", one=1))
    return col


def build(ctx, tc):
    nc = tc.nc

    x_in = nc.declare_dram_parameter("x", [NS, M, 3], F32, isOutput=False)
    wspec = {
        "fW1s": (T, 3, 64), "fb1": (T, 64),
        "fW2": (T, 64, C), "fb2": (T, C),
        "fW3c": (T, C, CH), "fb3c": (T, CH),
        "gb1": (T, 64),
        "gW2": (T, 64, 32), "gb2": (T, 32),
        "gW3": (T, 32, 3), "gb3": (T, 3),
    }
    w = {
        name: nc.declare_dram_parameter(name, list(shp), F32, isOutput=False)
        for name, shp in wspec.items()
    }
    w["gW1"] = nc.declare_dram_parameter("gW1", [T, C, 64], F16, isOutput=False)
    out_ext = nc.declare_dram_parameter("out", [NS, M, 3], F32, isOutput=True)

    agg_out = [[nc.dram_tensor(f"agg_out_t{t}s{s}", [CH, M], F16)
                for s in range(NS)] for t in range(T)]
    agg_full = [[nc.dram_tensor(f"agg_full_t{t}s{s}", [4, CH, M], F16)
                 for s in range(NS)] for t in range(T)]

    consts = ctx.enter_context(tc.tile_pool(name="consts", bufs=1))
    scratch_pool = ctx.enter_context(tc.tile_pool(name="scratch", bufs=3))
    work = ctx.enter_context(tc.tile_pool(name="work", bufs=2))
    ebc_pool = ctx.enter_context(tc.tile_pool(name="ebc", bufs=3))
    mg_pool = ctx.enter_context(tc.tile_pool(name="mg", bufs=2))
    psum = ctx.enter_context(
        tc.tile_pool(name="psum", bufs=2, space=bass.MemorySpace.PSUM)
    )
    psum_bc = ctx.enter_context(
        tc.tile_pool(name="psum_bc", bufs=2, space=bass.MemorySpace.PSUM)
    )
    psum_g = ctx.enter_context(
        tc.tile_pool(name="psum_g", bufs=1, space=bass.MemorySpace.PSUM)
    )

    identity = consts.tile([P, P], F32, tag="identity")
    make_identity(nc, identity)
    identity16 = consts.tile([P, P], F16, tag="identity16")
    make_identity(nc, identity16)

    def sel16(c, k):
        col = identity16[:k, c:c + 1]
        return bass.AP(col.tensor, col.offset, [list(col.ap[0]), [0, P]])

    # ---- weight/bias tiles ----
    wt = {}
    for t in range(T):
        tl = consts.tile([C, 64], F16, tag=f"gW1_{t}", name=f"gW1_{t}")
        nc.sync.dma_start(out=tl, in_=w["gW1"][t])
        wt[("gW1", t)] = tl
        for name, shp in wspec.items():
            if len(shp) == 3:
                tl = consts.tile([shp[1], shp[2]], F32, tag=f"{name}{t}",
                                 name=f"{name}{t}")
                nc.sync.dma_start(out=tl, in_=w[name][t])
            else:
                tl = _load_col(nc, consts, w[name][t], shp[1], f"{name}{t}")
            wt[(name, t)] = tl

    # ---- per-stream x load + transpose ----
    xTs, xns = [], []
    for s in range(NS):
        xn = []
        xT = consts.tile([3, M], F32, tag=f"xT{s}", name=f"xT{s}")
        for ib in range(NB):
            xn_ib = consts.tile([P, 3], F32, tag=f"xn{s}_{ib}",
                                name=f"xn{s}_{ib}")
            nc.sync.dma_start(out=xn_ib, in_=x_in[s, ib * P:(ib + 1) * P, :])
            xn.append(xn_ib)
            ps = psum.tile([3, P], F32, tag="aux", name=f"xt_ps{s}_{ib}")
            nc.tensor.transpose(ps, xn_ib, identity)
            nc.scalar.copy(out=xT[:, ib * P:(ib + 1) * P], in_=ps)
        xTs.append(xT)
        xns.append(xn)

    # adjacency tiles: one (P, NB, M) fp16 tile per stream; masked-max ops
    # broadcast it across the G channel-group dim with 0-stride APs.
    a16s = [consts.tile([P, NB, M], F16, tag=f"a16_{s}", name=f"a16_{s}")
            for s in range(NS)]

    def adjacency(s):
        """diff-based (not Gram) to dodge cancellation near R."""
        bcx = []
        for d in range(3):
            ps = psum.tile([P, M], F32, tag="aux", name=f"bcx_ps{s}_{d}")
            col = identity[:3, d:d + 1]
            sel3 = bass.AP(col.tensor, col.offset,
                           [list(col.ap[0]), [0, P]])
            nc.tensor.matmul(ps, sel3, xTs[s], start=True, stop=True)
            b = scratch_pool.tile([P, M], F32, tag="bcx", name=f"bcx{s}_{d}")
            nc.scalar.copy(out=b, in_=ps)
            bcx.append(b)
        for ib in range(NB):
            acc = scratch_pool.tile([P, M], F32, tag="adj_acc")
            for d in range(3):
                dif = scratch_pool.tile([P, M], F32, tag="adj_dif")
                nc.vector.tensor_scalar(
                    out=dif, in0=bcx[d], scalar1=xns[s][ib][:, d:d + 1],
                    scalar2=None, op0=ALU.subtract,
                )
                if d == 0:
                    nc.vector.tensor_mul(acc, dif, dif)
                else:
                    sq = scratch_pool.tile([P, M], F32, tag="adj_sq")
                    nc.vector.tensor_mul(sq, dif, dif)
                    nc.vector.tensor_add(acc, acc, sq)
            nc.vector.tensor_scalar(
                out=a16s[s][:, ib, :], in0=acc, scalar1=R, scalar2=None,
                op0=ALU.is_lt,
            )

    states = list(xTs)

    def mlp_layer(rhs, wname, bname, t, ndim, relu=True, out_dtype=F32,
                  nm=""):
        ps = psum.tile([ndim, M], F32, tag="mlp", name=f"ps_{nm}")
        nc.tensor.matmul(ps, wt[(wname, t)], rhs, start=True, stop=True)
        o = work.tile([ndim, M], out_dtype, tag=f"act_{wname}", name=nm)
        nc.scalar.activation(
            out=o, in_=ps, func=AF.Relu if relu else AF.Identity,
            bias=wt[(bname, t)], scale=1.0,
        )
        return o

    aggblks = {}

    def compute_mm(s, t):
        """edge MLP + broadcast + masked max (the DVE phase)."""
        stateT = states[s]
        h1T = mlp_layer(stateT, "fW1s", "fb1", t, 64, nm=f"h1_{s}_{t}")
        h2T = mlp_layer(h1T, "fW2", "fb2", t, C, nm=f"h2_{s}_{t}")
        ET = mlp_layer(h2T, "fW3c", "fb3c", t, CH, relu=False,
                       out_dtype=F16, nm=f"ET_{s}_{t}")

        a16 = a16s[s]
        adj_bc = bass.AP(a16.tensor, a16.offset,
                         [list(a16.ap[0]), list(a16.ap[1]), [0, G],
                          list(a16.ap[2])])
        aggblk = work.tile([P, NB, CH], F16, tag="aggblk",
                           name=f"aggblk{s}_{t}")
        for cg in range(NG):
            ebcg = ebc_pool.tile([P, G, M], F16, tag="ebcg")
            for cc in range(G):
                ps = psum_bc.tile([P, M], F32, tag="ebc",
                                  name=f"ebc{t}_{s}_{cg}_{cc}")
                nc.tensor.matmul(ps, sel16(cg * G + cc, CH), ET,
                                 start=True, stop=True)
                nc.scalar.copy(out=ebcg[:, cc, :], in_=ps)
            ebc_bc = bass.AP(ebcg.tensor, ebcg.offset,
                             [list(ebcg.ap[0]), [0, NB], list(ebcg.ap[1]),
                              list(ebcg.ap[2])])
            mg = mg_pool.tile([P, NB, G, M], F16, tag="mgrp")
            mg2 = mg_pool.tile([P, NB, G, M // 2], F16, tag="mgrp2")
            nc.vector.tensor_tensor(out=mg, in0=adj_bc, in1=ebc_bc,
                                    op=ALU.mult)
            nc.vector.tensor_tensor(
                out=mg2, in0=mg[:, :, :, :192], in1=mg[:, :, :, 192:],
                op=ALU.max)
            nc.vector.tensor_tensor(
                out=mg[:, :, :, :96], in0=mg2[:, :, :, :96],
                in1=mg2[:, :, :, 96:], op=ALU.max)
            nc.vector.tensor_tensor(
                out=mg2[:, :, :, :48], in0=mg[:, :, :, :48],
                in1=mg[:, :, :, 48:96], op=ALU.max)
            nc.vector.tensor_tensor(
                out=mg[:, :, :, :24], in0=mg2[:, :, :, :24],
                in1=mg2[:, :, :, 24:48], op=ALU.max)
            nc.vector.tensor_tensor(
                out=mg2[:, :, :, :12], in0=mg[:, :, :, :12],
                in1=mg[:, :, :, 12:24], op=ALU.max)
            nc.vector.tensor_tensor(
                out=mg[:, :, :, :6], in0=mg2[:, :, :, :6],
                in1=mg2[:, :, :, 6:12], op=ALU.max)
            nc.vector.tensor_reduce(
                out=aggblk[:, :, cg * G:(cg + 1) * G],
                in_=mg[:, :, :, :6], axis=AX.X, op=ALU.max,
            )
        aggblks[s] = aggblk

    def compute_fin(s, t):
        """transpose agg to (CH, M), store, launch AllGather."""
        aggblk = aggblks[s]
        aggT = work.tile([CH, M], F16, tag="aggT", name=f"aggT{s}_{t}")
        for ib in range(NB):
            ps = psum.tile([CH, P], F16, tag="aux", name=f"tr_agg{t}_{s}_{ib}")
            nc.tensor.transpose(ps, aggblk[:, ib, :], identity16)
            nc.scalar.copy(out=aggT[:, ib * P:(ib + 1) * P], in_=ps)
        nc.sync.dma_start(out=agg_out[t][s][:], in_=aggT)
        nc.gpsimd.collective_compute(
            "AllGather", ALU.bypass, replica_groups=REPLICA_GROUPS,
            ins=[agg_out[t][s][:]], outs=[agg_full[t][s][:]],
        )

    def g_phase(s, t):
        """gather in, MLP_g; the +state residual is accumulated into the
        g3 PSUM by an identity matmul so no DVE op is involved."""
        aggF = work.tile([C, M], F16, tag=f"aggF{s}",
                         name=f"aggF{t}_{s}")
        nc.sync.dma_start(
            out=aggF,
            in_=agg_full[t][s][:].rearrange("r c m -> (r c) m"))
        ps_g1 = psum_g.tile([64, M], F32, tag=f"psg1_{s}", name=f"psg1_{s}{t}")
        nc.tensor.matmul(ps_g1, wt[("gW1", t)], aggF, start=True, stop=True)
        g1T = work.tile([64, M], F32, tag="g1T", name=f"g1T_{s}_{t}")
        nc.scalar.activation(out=g1T, in_=ps_g1, func=AF.Relu,
                             bias=wt[("gb1", t)], scale=1.0)
        g2T = mlp_layer(g1T, "gW2", "gb2", t, 32, nm=f"g2_{s}_{t}")
        gdT = mlp_layer(g2T, "gW3", "gb3", t, 3, nm=f"g3_{s}_{t}")
        # state residual via identity-matmul accumulation (keeps the add
        # off the vector engine; the relu above must precede the add)
        ps_n = psum.tile([3, M], F32, tag="mlp", name=f"ps_n_{s}_{t}")
        nc.tensor.matmul(ps_n, identity[:3, :3], gdT, start=True, stop=False)
        nc.tensor.matmul(ps_n, identity[:3, :3], states[s], start=False,
                         stop=True)
        newT = work.tile([3, M], F32, tag=f"stateT{s}", name=f"stateT{s}_{t}")
        nc.scalar.copy(out=newT, in_=ps_n)
        states[s] = newT

    def out_phase(s):
        for ib in range(NB):
            ps = psum.tile([P, 3], F32, tag="aux", name=f"tr_out{s}_{ib}")
            nc.tensor.transpose(ps, states[s][:, ib * P:(ib + 1) * P],
                                identity[:3, :3])
            o = work.tile([P, 3], F32, tag="out_sb", name=f"out_sb{s}_{ib}")
            nc.scalar.copy(out=o, in_=ps)
            nc.sync.dma_start(out=out_ext[s, ib * P:(ib + 1) * P, :], in_=o)

    # ---- software-pipelined schedule ----
    # A_mm = compute_mm (DVE-heavy), A_fin = compute_fin, B = g_phase.
    # Steady state: B(s,t) + A_mm(s,t+1)'s tensor/scalar head run under
    # the OTHER stream's A_mm DVE work.
    adjacency(0)
    compute_mm(0, 0)
    adjacency(1)
    compute_fin(0, 0)
    compute_mm(1, 0)
    for t in range(T):
        g_phase(0, t)
        if t == T - 1:
            out_phase(0)
        else:
            compute_mm(0, t + 1)
        compute_fin(1, t)
        g_phase(1, t)
        if t == T - 1:
            out_phase(1)
        else:
            compute_mm(1, t + 1)
            compute_fin(0, t + 1)


_NC_CACHE = None


def _build_nc():
    global _NC_CACHE
    if _NC_CACHE is None:
        nc = bacc.Bacc(
            "TRN2", target_bir_lowering=False, debug=False,
            num_devices=N_CORES,
        )
        with ExitStack() as ctx:
            tc = ctx.enter_context(tile.TileContext(nc))
            build(ctx, tc)
        nc.compile()
        _NC_CACHE = nc
    return _NC_CACHE


def _in_maps(inputs):
    maps = []
    fW1s = np.ascontiguousarray(inputs["fW1"][:, 3:6, :])
    for k in range(N_CORES):
        q, r = k // 4, k % 4
        sl = slice(CH * r, CH * r + CH)
        maps.append({
            "x": np.ascontiguousarray(inputs["x"][2 * q:2 * q + 2]),
            "fW1s": fW1s,
            "fb1": inputs["fb1"],
            "fW2": inputs["fW2"],
            "fb2": inputs["fb2"],
            "fW3c": np.ascontiguousarray(inputs["fW3"][:, :, sl]),
            "fb3c": np.ascontiguousarray(inputs["fb3"][:, sl]),
            "gW1": inputs["gW1"].astype(np.float16),
            "gb1": inputs["gb1"],
            "gW2": inputs["gW2"],
            "gb2": inputs["gb2"],
            "gW3": inputs["gW3"],
            "gb3": inputs["gb3"],
        })
    return maps


def kernel(trace=False, **inputs):
    _register_ntff_hook()
    nc = _build_nc()
    inputs = {k: np.asarray(v, np.float32) for k, v in inputs.items()}
    res = run_bass_kernel_spmd(
        nc, _in_maps(inputs), list(range(N_CORES)), trace=trace,
    )
    out = np.stack([res.results[4 * (f // 2)]["out"][f % 2]
                    for f in range(N_FRAMES)])
    if trace:
        kernel.last_results = res
    return out.astype(np.float32)
